# revision 1
# baseline (speedup 1.0000x reference)
"""Trainium2 Bass kernel for nn_BERTEmbedding (fused per-index affine + sinusoidal PE).

Math (per batch b, vocab-position v, embed index e):
    out[b,v,e] = s0[b,v]*flux_w[v,e] + flux_b[v,e]
               + s2[b,v]*time_w[v,e] + time_b[v,e]
               + (e even: sin(s1[b,v]*div[e/2]) ; e odd: cos(s1[b,v]*div[(e-1)/2]))

Sharding: vocab axis V=4096 split across 8 cores (512 rows each); every core
handles all 16 batches of its vocab shard.  The weight tables are sharded with
the vocab axis, so each core only ever reads its 512-row slices.

Device strategy (per core, 4 v-tiles x 16 batches = 64 work items of [128,768]):
  - TensorE: psum = diag(s0) @ fw + diag(s2) @ tw + I @ bsum   (float32r matmuls,
    diagonal-scaling trick; bsum = flux_b + time_b folded on host)
  - ScalarE: pe tile via Sin activation, laid out as [sin half | cos half] so
    every ACT write is contiguous.  ScalarE's Sin is only valid on [-pi, pi]:
      k >= KLO: |s1|*d_KLO + pi/2 < pi for this problem's inputs -> direct Sin
                with per-partition scale=s1
      k <  KLO: host ships integer phase codes combo_n[b,v,:] (bf16-exact):
                arg*(2/pi) = s1*dv2' + combo_n with dv2' = d_k*2/pi, where
                combo_n = j - 4*rint((s1*d_k + j*pi/2)/2pi), j in {0,1}.
                One fused DVE op builds r' and one Sin(scale=pi/2) evaluates it.
  - VectorE: builds diag tiles (tensor_scalar on identity), the lo-lane fused
    angle op, and the single merge out = psum + pe (interleaving sin/cos via
    the read access pattern; also evacuates PSUM)
  - DMA: table + combo loads once per v-tile; one 393KB store per work item
"""

import math

import numpy as np

try:
    import concourse.bass as bass
except ImportError:  # harness containers keep the repo at /opt/trn_rl_repo
    import sys

    sys.path.insert(0, "/opt/trn_rl_repo")
    import concourse.bass as bass

import concourse.bacc as bacc
import concourse.tile as tile
from concourse import mybir
from concourse.bass_utils import run_bass_kernel_spmd

B, V, E = 16, 4096, 768
EH = E // 2  # 384 angle lanes
KLO = 48  # angle lanes fixed up via the host combo tensor
N_CORES = 8
V_SHARD = V // N_CORES  # 512
VT = V_SHARD // 128  # 4 v-tiles per core
F32 = mybir.dt.float32
F32R = mybir.dt.float32r
BF16 = mybir.dt.bfloat16

TWO_PI = 2.0 * math.pi
HALF_PI = float(np.float32(math.pi / 2.0))
# keep reduced angles strictly inside ScalarE's [-pi, pi] spline domain
SIN_SAFETY = 1.0 - 1e-6
# direct-Sin lanes need |s1|*d_KLO + pi/2 <= pi
S1_LIMIT = (math.pi / 2.0) / math.exp(-KLO * math.log(10000.0) / EH)


def build_bass() -> "bass.Bass":
    from contextlib import ExitStack

    nc = bacc.Bacc(
        "TRN2",
        target_bir_lowering=False,
        debug=False,
        num_devices=N_CORES,
    )
    Alu = mybir.AluOpType

    seq_d = nc.dram_tensor("seq", [128, VT * B * 3], F32, kind="ExternalInput")
    fw_d = nc.dram_tensor("fw", [V_SHARD, E], F32R, kind="ExternalInput")
    tw_d = nc.dram_tensor("tw", [V_SHARD, E], F32R, kind="ExternalInput")
    bs_d = nc.dram_tensor("bs", [V_SHARD, E], F32R, kind="ExternalInput")
    dv_d = nc.dram_tensor("dv", [128, EH], F32, kind="ExternalInput")
    dv2_d = nc.dram_tensor("dv2lo", [128, 2 * KLO], F32, kind="ExternalInput")
    cmb_d = nc.dram_tensor("combo", [128, VT * B * 2 * KLO], BF16, kind="ExternalInput")
    eye_d = nc.dram_tensor("eye", [128, 128], F32R, kind="ExternalInput")
    out_d = nc.dram_tensor("out", [B, V_SHARD, E], F32, kind="ExternalOutput")

    with tile.TileContext(nc) as tc, ExitStack() as ctx:
        const_pool = ctx.enter_context(tc.tile_pool(name="const", bufs=1))
        tab_pool = ctx.enter_context(tc.tile_pool(name="tables", bufs=2))
        diag_pool = ctx.enter_context(tc.tile_pool(name="diag", bufs=6))
        ang_pool = ctx.enter_context(tc.tile_pool(name="ang", bufs=6))
        pe_pool = ctx.enter_context(tc.tile_pool(name="pe", bufs=6))
        out_pool = ctx.enter_context(tc.tile_pool(name="out", bufs=6))
        psum_pool = ctx.enter_context(tc.tile_pool(name="psum", bufs=4, space="PSUM"))

        zero_t = const_pool.tile([128, 1], F32, tag="zero")
        nc.vector.memset(zero_t[:], 0.0)
        hpi_t = const_pool.tile([128, 1], F32, tag="hpi")
        nc.vector.memset(hpi_t[:], HALF_PI)

        seq_t = const_pool.tile([128, VT * B * 3], F32, tag="seq")
        nc.sync.dma_start(seq_t[:], seq_d[:])
        dv_t = const_pool.tile([128, EH], F32, tag="dv")
        nc.sync.dma_start(dv_t[:], dv_d[:])
        dv2_t = const_pool.tile([128, 2 * KLO], F32, tag="dv2")
        nc.sync.dma_start(dv2_t[:], dv2_d[:])
        eye_t = const_pool.tile([128, 128], F32R, tag="eye")
        nc.sync.dma_start(eye_t[:], eye_d[:])

        for vt in range(VT):
            fw_t = tab_pool.tile([128, E], F32R, tag="fw")
            nc.sync.dma_start(fw_t[:], fw_d[vt * 128 : (vt + 1) * 128, :])
            tw_t = tab_pool.tile([128, E], F32R, tag="tw")
            nc.sync.dma_start(tw_t[:], tw_d[vt * 128 : (vt + 1) * 128, :])
            bs_t = tab_pool.tile([128, E], F32R, tag="bs")
            nc.sync.dma_start(bs_t[:], bs_d[vt * 128 : (vt + 1) * 128, :])
            cmb_t = tab_pool.tile([128, B * 2 * KLO], BF16, tag="cmb")
            nc.sync.dma_start(
                cmb_t[:], cmb_d[:, vt * B * 2 * KLO : (vt + 1) * B * 2 * KLO]
            )

            GB = 4  # batches per pe group (amortizes ACT per-op overhead)
            KHI = EH - KLO  # 336 direct sin lanes
            for g in range(B // GB):
                bs4 = range(g * GB, (g + 1) * GB)

                # group staging: pre-scaled hi angles (GPSIMD) + lo codes (DVE)
                ang4 = ang_pool.tile([128, GB * KHI], F32, tag="ang4")
                r4 = ang_pool.tile([128, GB * 2 * KLO], F32, tag="r4")
                # pe group layout: per b, [ sin(0:384) | cos(384:768) ]
                pe4 = pe_pool.tile([128, GB * E], F32, tag="pe4")
                for i, b in enumerate(bs4):
                    col = vt * B * 3 + b * 3
                    s1 = seq_t[:, col + 1 : col + 2]
                    nc.gpsimd.tensor_tensor(
                        ang4[:, i * KHI : (i + 1) * KHI],
                        dv_t[:, KLO:EH],
                        s1.broadcast_to((128, KHI)),
                        Alu.mult,
                    )
                    nc.vector.scalar_tensor_tensor(
                        r4[:, i * 2 * KLO : (i + 1) * 2 * KLO],
                        dv2_t[:],
                        s1,
                        cmb_t[:, b * 2 * KLO : (b + 1) * 2 * KLO],
                        Alu.mult,
                        Alu.add,
                    )

                # batched Sin ops covering the whole group
                nc.scalar.activation(
                    pe4[:].rearrange("p (i e) -> p i e", i=GB)[:, :, KLO:EH],
                    ang4[:].rearrange("p (i k) -> p i k", i=GB),
                    mybir.ActivationFunctionType.Sin,
                    bias=zero_t[:],
                    scale=1.0,
                )
                nc.scalar.activation(
                    pe4[:].rearrange("p (i e) -> p i e", i=GB)[:, :, EH + KLO : E],
                    ang4[:].rearrange("p (i k) -> p i k", i=GB),
                    mybir.ActivationFunctionType.Sin,
                    bias=hpi_t[:],
                    scale=1.0,
                )
                # lo block: first 48 -> sin half start, next 48 -> cos half start
                nc.scalar.activation(
                    pe4[:]
                    .rearrange("p (i h q) -> p i h q", i=GB, h=2)[:, :, :, 0:KLO],
                    r4[:].rearrange("p (i h q) -> p i h q", i=GB, h=2),
                    mybir.ActivationFunctionType.Sin,
                    bias=zero_t[:],
                    scale=HALF_PI * SIN_SAFETY,
                )

                for i, b in enumerate(bs4):
                    col = vt * B * 3 + b * 3
                    s0 = seq_t[:, col : col + 1]
                    s2 = seq_t[:, col + 2 : col + 3]

                    # diag builds: d0 on ScalarE (Copy with per-row scale),
                    # d2 on GPSIMD - DVE keeps only the merge + lo codes
                    d0 = diag_pool.tile([128, 128], F32R, tag="d0")
                    nc.scalar.mul(d0[:], eye_t[:], s0)
                    d2 = diag_pool.tile([128, 128], F32R, tag="d2")
                    nc.gpsimd.tensor_tensor(
                        d2[:],
                        eye_t[:],
                        s2.broadcast_to((128, 128)).bitcast(F32R),
                        Alu.mult,
                    )

                    # psum = diag(s0)@fw + diag(s2)@tw + I@bsum, split 512/256
                    # to keep each matmul inside one PSUM bank
                    ps = psum_pool.tile([128, E], F32, tag="ps")
                    A, Bx = (0, 512), (512, E)
                    for w, t in ((d0[:], fw_t), (d2[:], tw_t)):
                        for lo, hi in (A, Bx):
                            nc.tensor.matmul(
                                ps[:, lo:hi],
                                w,
                                t[:, lo:hi],
                                start=t is fw_t,
                                stop=False,
                            )
                    for lo, hi in (A, Bx):
                        nc.tensor.matmul(
                            ps[:, lo:hi],
                            eye_t[:],
                            bs_t[:, lo:hi],
                            start=False,
                            stop=True,
                        )

                    # single merge; interleaves sin/cos via the read pattern
                    o_t = out_pool.tile([128, E], F32, tag="o")
                    nc.vector.tensor_add(
                        o_t[:].rearrange("p (q j) -> p q j", j=2),
                        ps[:].rearrange("p (q j) -> p q j", j=2),
                        pe4[:, i * E : (i + 1) * E].rearrange(
                            "p (j q) -> p q j", j=2
                        ),
                    )

                    nc.sync.dma_start(
                        out_d[b, vt * 128 : (vt + 1) * 128, :], o_t[:]
                    )

    nc.finalize()
    return nc


_NC_CACHE: list = []


def _get_nc():
    if not _NC_CACHE:
        _NC_CACHE.append(build_bass())
    return _NC_CACHE[0]


def make_in_maps(sequence, flux_w, flux_b, time_w, time_b):
    import ml_dtypes

    sequence = np.asarray(sequence, dtype=np.float32)
    flux_w = np.asarray(flux_w, dtype=np.float32)
    time_w = np.asarray(time_w, dtype=np.float32)
    bsum = np.asarray(flux_b, dtype=np.float32) + np.asarray(time_b, dtype=np.float32)

    s1_all = sequence[:, :, 1]
    assert np.abs(s1_all).max() < S1_LIMIT, (
        f"positional channel exceeds direct-Sin range: {np.abs(s1_all).max():.3f} "
        f">= {S1_LIMIT:.3f}; raise KLO"
    )

    div = np.exp(
        np.arange(0, E, 2, dtype=np.float32) * np.float32(-math.log(10000.0) / E)
    ).astype(np.float32)
    dv_rep = np.ascontiguousarray(np.broadcast_to(div, (128, EH)))
    # lo block: [48 sin lanes | 48 cos lanes], scaled by 2/pi
    dv2p = (np.concatenate([div[:KLO], div[:KLO]]) * np.float32(2.0 / math.pi)).astype(
        np.float32
    )
    dv2_lo = np.ascontiguousarray(np.broadcast_to(dv2p, (128, 2 * KLO)))
    eye = np.eye(128, dtype=np.float32)

    # combo_n[b,v,h*KLO+k] = j - 4*rint((s1*d_k + j*pi/2)/2pi), j = h (0=sin,1=cos)
    jj = np.concatenate([np.zeros(KLO, np.float64), np.ones(KLO, np.float64)])
    dd = np.concatenate([div[:KLO], div[:KLO]]).astype(np.float64)
    ang = s1_all[:, :, None].astype(np.float64) * dd[None, None, :] + jj * (
        math.pi / 2.0
    )
    n = np.rint(ang / TWO_PI)
    combo_n = (jj[None, None, :] - 4.0 * n).astype(np.float32)
    assert np.abs(combo_n).max() <= 16, "combo codes exceed bf16-exact range"
    combo_bf = combo_n.astype(ml_dtypes.bfloat16)  # small ints: bf16-exact

    in_maps = []
    for c in range(N_CORES):
        v0, v1 = c * V_SHARD, (c + 1) * V_SHARD
        # [B, 512, 3] -> [128p, vt*B*3 + b*3 + ch]
        s = sequence[:, v0:v1, :].reshape(B, VT, 128, 3)
        seq_r = np.ascontiguousarray(s.transpose(2, 1, 0, 3)).reshape(128, VT * B * 3)
        # combo [B, 512, 2*KLO] -> [128p, (vt*B + b)*2*KLO + lane]
        cmb = combo_bf[:, v0:v1, :].reshape(B, VT, 128, 2 * KLO)
        cmb_r = np.ascontiguousarray(cmb.transpose(2, 1, 0, 3)).reshape(
            128, VT * B * 2 * KLO
        )
        in_maps.append(
            {
                "seq": seq_r,
                "fw": np.ascontiguousarray(flux_w[v0:v1]),
                "tw": np.ascontiguousarray(time_w[v0:v1]),
                "bs": np.ascontiguousarray(bsum[v0:v1]),
                "dv": dv_rep,
                "dv2lo": dv2_lo,
                "combo": cmb_r,
                "eye": eye,
            }
        )
    return in_maps


def run(in_maps, trace: bool = False):
    nc = _get_nc()
    return run_bass_kernel_spmd(nc, in_maps, list(range(N_CORES)), trace=trace)


def kernel(sequence, flux_w, flux_b, time_w, time_b) -> np.ndarray:
    in_maps = make_in_maps(sequence, flux_w, flux_b, time_w, time_b)
    res = run(in_maps)
    out = np.concatenate([res.results[c]["out"] for c in range(N_CORES)], axis=1)
    return np.ascontiguousarray(out.astype(np.float32, copy=False))



# revision 9
# speedup vs baseline: 1.1602x; 1.1602x over previous
"""Trainium2 Bass kernel for nn_BERTEmbedding (fused per-index affine + sinusoidal PE).

Math (per batch b, vocab-position v, embed index e):
    out[b,v,e] = s0[b,v]*flux_w[v,e] + flux_b[v,e]
               + s2[b,v]*time_w[v,e] + time_b[v,e]
               + (e even: sin(s1[b,v]*div[e/2]) ; e odd: cos(s1[b,v]*div[(e-1)/2]))

Sharding: vocab axis V=4096 split across 8 cores (512 rows each); every core
handles all 16 batches of its vocab shard.  Weight tables are sharded with the
vocab axis and shipped in bf16 (halves table DMA; ~0.2% relative error on the
small affine terms, far under the 2e-2 gate).

Device strategy (per core, 4 v-tiles x 16 batches = 64 items of [128,768]):
  - TensorE: psum = diag(s0) @ fw + diag(s2) @ tw + I @ bsum   (bf16 matmuls;
    bsum = flux_b + time_b folded on host)
  - ScalarE: all sin/cos evals, batched 16 batches per ACTIVATE (amortizes the
    ~352-cycle per-op overhead).  ScalarE Sin is valid on [-pi, pi]:
      k >= KLO: |s1|*d_KLO + pi/2 < pi for this problem -> angles staged on
                GpSimd (ang = s1 * dv, per-partition tensor_scalar)
      k <  KLO: host ships fully wrapped+clipped angles in fp16 (alo); ACT
                reads them directly -- no on-device range reduction at all.
    ScalarE also builds the d0 diag (Copy with per-partition scale).
  - GpSimd: angle staging + d2 diag (tensor_scalar ops; GpSimd never touches
    2-port DVE modes so there is no SBUF port contention).
  - VectorE: ONLY the psum+pe merges (tensor_tensor, 1x mode, never contends),
    batched 2 items per op; interleaves sin/cos via the read access pattern.
  - DMA: all loads prefetched up front; stores batched 2 items (786KB) each.

Engine budget (predicted, warm): ACT ~70us, DVE ~61, GpSimd ~54, PE ~61,
DMA wire ~82-86us (29.4MB @ ~358GB/s) -> ~90-96us total.
"""

import math

import numpy as np

try:
    import concourse.bass as bass
except ImportError:  # harness containers keep the repo at /opt/trn_rl_repo
    import sys

    sys.path.insert(0, "/opt/trn_rl_repo")
    import concourse.bass as bass

import concourse.bacc as bacc
import concourse.tile as tile
from concourse import mybir
from concourse.bass_utils import run_bass_kernel_spmd

B, V, E = 16, 4096, 768
EH = E // 2  # 384 angle lanes
KLO = 48  # angle lanes shipped pre-wrapped from host
KHI = EH - KLO  # 336 direct-Sin lanes
N_CORES = 8
V_SHARD = V // N_CORES  # 512
VT = V_SHARD // 128  # 4 v-tiles per core
F32 = mybir.dt.float32
BF16 = mybir.dt.bfloat16
FP16 = mybir.dt.float16

TWO_PI = 2.0 * math.pi
HALF_PI = float(np.float32(math.pi / 2.0))
# keep shipped lo angles strictly inside ScalarE's [-pi, pi] spline domain
ALO_CLIP = math.pi - 2e-3
# direct-Sin lanes need |s1|*d_KLO + pi/2 <= pi
S1_LIMIT = (math.pi / 2.0) / math.exp(-KLO * math.log(10000.0) / EH)


def build_bass() -> "bass.Bass":
    from contextlib import ExitStack

    nc = bacc.Bacc(
        "TRN2",
        target_bir_lowering=False,
        debug=False,
        num_devices=N_CORES,
    )

    Alu = mybir.AluOpType

    seq_d = nc.dram_tensor("seq", [128, VT * B * 3], F32, kind="ExternalInput")
    s2b_d = nc.dram_tensor("s2b", [128, VT * B], BF16, kind="ExternalInput")
    fw_d = nc.dram_tensor("fw", [V_SHARD, E], BF16, kind="ExternalInput")
    tw_d = nc.dram_tensor("tw", [V_SHARD, E], BF16, kind="ExternalInput")
    bs_d = nc.dram_tensor("bs", [V_SHARD, E], BF16, kind="ExternalInput")
    dv_d = nc.dram_tensor("dv", [128, KHI], F32, kind="ExternalInput")
    alo_d = nc.dram_tensor("alo", [128, VT * B * 2 * KLO], FP16, kind="ExternalInput")
    eye_d = nc.dram_tensor("eye", [128, 128], BF16, kind="ExternalInput")
    out_d = nc.dram_tensor("out", [B, V_SHARD, E], F32, kind="ExternalOutput")

    with tile.TileContext(nc) as tc, ExitStack() as ctx:
        const_pool = ctx.enter_context(tc.tile_pool(name="const", bufs=1))
        ang_pool = ctx.enter_context(tc.tile_pool(name="ang", bufs=2))
        pe_pool = ctx.enter_context(tc.tile_pool(name="pe", bufs=2))
        diag_pool = ctx.enter_context(tc.tile_pool(name="diag", bufs=6))
        out_pool = ctx.enter_context(tc.tile_pool(name="out", bufs=3))
        psum_pool = ctx.enter_context(tc.tile_pool(name="psum", bufs=2, space="PSUM"))

        zero_t = const_pool.tile([128, 1], F32, tag="zero")
        nc.vector.memset(zero_t[:], 0.0)
        hpi_t = const_pool.tile([128, 1], F32, tag="hpi")
        nc.vector.memset(hpi_t[:], HALF_PI)

        seq_t = const_pool.tile([128, VT * B * 3], F32, tag="seq")
        nc.sync.dma_start(seq_t[:], seq_d[:])
        s2b_t = const_pool.tile([128, VT * B], BF16, tag="s2b")
        nc.sync.dma_start(s2b_t[:], s2b_d[:])
        dv_t = const_pool.tile([128, KHI], F32, tag="dv")
        nc.sync.dma_start(dv_t[:], dv_d[:])
        eye_t = const_pool.tile([128, 128], BF16, tag="eye")
        nc.sync.dma_start(eye_t[:], eye_d[:])

        # prefetch every table up front; all stay resident in SBUF
        fw_ts, tw_ts, bs_ts, alo_ts = [], [], [], []
        for vt in range(VT):
            fw_t = const_pool.tile([128, E], BF16, tag=f"fw{vt}")
            nc.sync.dma_start(fw_t[:], fw_d[vt * 128 : (vt + 1) * 128, :])
            tw_t = const_pool.tile([128, E], BF16, tag=f"tw{vt}")
            nc.sync.dma_start(tw_t[:], tw_d[vt * 128 : (vt + 1) * 128, :])
            bs_t = const_pool.tile([128, E], BF16, tag=f"bs{vt}")
            nc.sync.dma_start(bs_t[:], bs_d[vt * 128 : (vt + 1) * 128, :])
            alo_t = const_pool.tile([128, B * 2 * KLO], FP16, tag=f"alo{vt}")
            nc.sync.dma_start(
                alo_t[:], alo_d[:, vt * B * 2 * KLO : (vt + 1) * B * 2 * KLO]
            )
            fw_ts.append(fw_t)
            tw_ts.append(tw_t)
            bs_ts.append(bs_t)
            alo_ts.append(alo_t)

        for vt in range(VT):
            fw_t, tw_t, bs_t, alo_t = fw_ts[vt], tw_ts[vt], bs_ts[vt], alo_ts[vt]

            # --- group staging: all 16 batches of this v-tile ---
            ang = ang_pool.tile([128, B * KHI], F32, tag="ang")
            for b in range(B):
                col = vt * B * 3 + b * 3
                s1 = seq_t[:, col + 1 : col + 2]
                nc.gpsimd.tensor_tensor(
                    ang[:, b * KHI : (b + 1) * KHI],
                    dv_t[:],
                    s1.broadcast_to((128, KHI)),
                    Alu.mult,
                )

            # pe layout per b: [ sin(0:384) | cos(384:768) ]
            pe = pe_pool.tile([128, B * E], BF16, tag="pe")
            pe_i = pe[:].rearrange("p (i e) -> p i e", i=B)
            ang_i = ang[:].rearrange("p (i k) -> p i k", i=B)
            nc.scalar.activation(
                pe_i[:, :, KLO:EH],
                ang_i,
                mybir.ActivationFunctionType.Sin,
                bias=zero_t[:],
                scale=1.0,
            )
            nc.scalar.activation(
                pe_i[:, :, EH + KLO : E],
                ang_i,
                mybir.ActivationFunctionType.Sin,
                bias=hpi_t[:],
                scale=1.0,
            )
            # lo lanes: host shipped wrapped angles; [48 sin | 48 cos] per b
            nc.scalar.activation(
                pe[:].rearrange("p (i h q) -> p i h q", i=B, h=2)[:, :, :, 0:KLO],
                alo_t[:].rearrange("p (i h q) -> p i h q", i=B, h=2),
                mybir.ActivationFunctionType.Sin,
                bias=zero_t[:],
                scale=1.0,
            )

            for pair in range(B // 2):
                ps = psum_pool.tile([128, 2048], F32, tag="ps")
                for j in (0, 1):
                    b = pair * 2 + j
                    col = vt * B * 3 + b * 3
                    s0 = seq_t[:, col : col + 1]

                    d0 = diag_pool.tile([128, 128], BF16, tag="d0")
                    nc.scalar.mul(d0[:], eye_t[:], s0)
                    d2 = diag_pool.tile([128, 128], BF16, tag="d2")
                    nc.gpsimd.tensor_tensor(
                        d2[:],
                        eye_t[:],
                        s2b_t[:, vt * B + b : vt * B + b + 1].broadcast_to((128, 128)),
                        Alu.mult,
                    )

                    # psum[:, j*1024 : j*1024+768] = d0@fw + d2@tw + I@bs
                    # (512/256 split keeps each matmul inside one PSUM bank)
                    off = j * 1024
                    for w, t in ((d0, fw_t), (d2, tw_t), (eye_t, bs_t)):
                        for lo, hi in ((0, 512), (512, E)):
                            nc.tensor.matmul(
                                ps[:, off + lo : off + hi],
                                w[:],
                                t[:, lo:hi],
                                start=t is fw_t,
                                stop=t is bs_t,
                            )

                # merge both items; interleave sin/cos via the read pattern
                out2 = out_pool.tile([128, 2 * E], F32, tag="o")
                nc.vector.tensor_add(
                    out2[:].rearrange("p (j q h) -> p j q h", j=2, h=2),
                    ps[:]
                    .rearrange("p (j x) -> p j x", j=2)[:, :, 0:E]
                    .rearrange("p j (q h) -> p j q h", h=2),
                    pe[:, pair * 2 * E : (pair * 2 + 2) * E].rearrange(
                        "p (j h q) -> p j q h", j=2, h=2
                    ),
                )

                nc.sync.dma_start(
                    out_d[pair * 2 : pair * 2 + 2, vt * 128 : (vt + 1) * 128, :]
                    .rearrange("j p e -> p j e"),
                    out2[:].rearrange("p (j e) -> p j e", j=2),
                )

    nc.finalize()
    return nc


_NC_CACHE: list = []


def _get_nc():
    if not _NC_CACHE:
        _NC_CACHE.append(build_bass())
    return _NC_CACHE[0]


def make_in_maps(sequence, flux_w, flux_b, time_w, time_b):
    import ml_dtypes

    sequence = np.asarray(sequence, dtype=np.float32)
    bsum = np.asarray(flux_b, dtype=np.float32) + np.asarray(time_b, dtype=np.float32)
    fw_bf = np.asarray(flux_w, dtype=np.float32).astype(ml_dtypes.bfloat16)
    tw_bf = np.asarray(time_w, dtype=np.float32).astype(ml_dtypes.bfloat16)
    bs_bf = bsum.astype(ml_dtypes.bfloat16)

    s1_all = sequence[:, :, 1]
    assert np.abs(s1_all).max() < S1_LIMIT, (
        f"positional channel exceeds direct-Sin range: {np.abs(s1_all).max():.3f} "
        f">= {S1_LIMIT:.3f}; raise KLO"
    )

    div = np.exp(
        np.arange(0, E, 2, dtype=np.float32) * np.float32(-math.log(10000.0) / E)
    ).astype(np.float32)
    dv_rep = np.ascontiguousarray(np.broadcast_to(div[KLO:], (128, KHI)))
    eye = np.eye(128, dtype=np.float32).astype(ml_dtypes.bfloat16)

    # lo lanes: fully wrapped angles, fp16.  alo[b,v,h*KLO+k] =
    # wrap(s1*d_k + h*pi/2) into (-pi, pi), clipped inside the spline domain.
    jj = np.concatenate([np.zeros(KLO, np.float64), np.ones(KLO, np.float64)])
    dd = np.concatenate([div[:KLO], div[:KLO]]).astype(np.float64)
    ang = s1_all[:, :, None].astype(np.float64) * dd[None, None, :] + jj * (
        math.pi / 2.0
    )
    wrapped = ang - TWO_PI * np.rint(ang / TWO_PI)
    alo = np.clip(wrapped, -ALO_CLIP, ALO_CLIP).astype(np.float16)

    in_maps = []
    for c in range(N_CORES):
        v0, v1 = c * V_SHARD, (c + 1) * V_SHARD
        # [B, 512, 3] -> [128p, vt*B*3 + b*3 + ch]
        s = sequence[:, v0:v1, :].reshape(B, VT, 128, 3)
        seq_r = np.ascontiguousarray(s.transpose(2, 1, 0, 3)).reshape(128, VT * B * 3)
        # s2 channel in bf16 for the all-bf16 d2 diag build on GpSimd
        s2b_r = np.ascontiguousarray(seq_r[:, 2::3]).astype(ml_dtypes.bfloat16)
        # alo [B, 512, 2*KLO] -> [128p, (vt*B + b)*2*KLO + lane]
        a = alo[:, v0:v1, :].reshape(B, VT, 128, 2 * KLO)
        alo_r = np.ascontiguousarray(a.transpose(2, 1, 0, 3)).reshape(
            128, VT * B * 2 * KLO
        )
        in_maps.append(
            {
                "seq": seq_r,
                "s2b": s2b_r,
                "fw": np.ascontiguousarray(fw_bf[v0:v1]),
                "tw": np.ascontiguousarray(tw_bf[v0:v1]),
                "bs": np.ascontiguousarray(bs_bf[v0:v1]),
                "dv": dv_rep,
                "alo": alo_r,
                "eye": eye,
            }
        )
    return in_maps


def run(in_maps, trace: bool = False):
    nc = _get_nc()
    return run_bass_kernel_spmd(nc, in_maps, list(range(N_CORES)), trace=trace)


def kernel(sequence, flux_w, flux_b, time_w, time_b) -> np.ndarray:
    in_maps = make_in_maps(sequence, flux_w, flux_b, time_w, time_b)
    res = run(in_maps)
    out = np.concatenate([res.results[c]["out"] for c in range(N_CORES)], axis=1)
    return np.ascontiguousarray(out.astype(np.float32, copy=False))
